# revision 14
# baseline (speedup 1.0000x reference)
"""CapsuleLayer dynamic-routing kernel for 8 Trainium2 NeuronCores.

Sharding: data-parallel over batch (16 batches/core), weight replicated.
  u_hat[b,c,n,s] = sum_i W[c,n,s,i] * x[b,i,c]   (PE, fp32r, block-diag x)
  3 routing iterations; the b_ij update takes a mean over the full batch
  via one AllReduce per iteration (skipped on the last).

On-chip: u_hat kept in SBUF as A[c%128, chunk, b, n, s] (9 chunks of 128
channels).  s_j is a PE pass (c_ij stationary, u_hat moving); the
agreement <u_hat, v> is a GPSIMD multiply + DVE segmented reduce.
"""

import sys

sys.path.insert(0, "/opt/trn_rl_repo")

import numpy as np

B, IN_UNIT, IN_CHANNEL = 128, 16, 1152
NUM_UNIT, UNIT_SIZE = 16, 16
NCORES = 8
BL = B // NCORES               # 16 batches per core
NGROUP = IN_CHANNEL // 8       # 144 groups of 8 channels
NCHUNK = IN_CHANNEL // 128     # 9 c-chunks
NS = NUM_UNIT * UNIT_SIZE      # 256
FREE = BL * NS                 # 4096 = (b, n, s) free size per chunk

_cache = {}


def _build():
    import concourse.bass as bass
    import concourse.bacc as bacc
    import concourse.mybir as mybir
    import concourse.tile as tile

    f32 = mybir.dt.float32
    f32r = mybir.dt.float32r
    ALU = mybir.AluOpType
    AX = mybir.AxisListType

    def sub(ap, off, dims, cast=None):
        a = bass.AP(ap.tensor, ap.offset + off, [list(d) for d in dims])
        return a.bitcast(cast) if cast is not None else a

    nc = bacc.Bacc("TRN2", target_bir_lowering=False, debug=False,
                   num_devices=NCORES)

    wr_t = nc.dram_tensor("wr", [NGROUP * 128, 256], f32, kind="ExternalInput")
    xc_t = nc.dram_tensor("xc", [IN_CHANNEL, IN_UNIT, BL], f32, kind="ExternalInput")
    cij1_t = nc.dram_tensor("cij1", [128, NUM_UNIT], f32, kind="ExternalInput")
    vout_t = nc.dram_tensor("vout", [NUM_UNIT, BL * UNIT_SIZE], f32,
                            kind="ExternalOutput")

    with tile.TileContext(nc) as tc:
        with tc.tile_pool(name="apool", bufs=1) as apool, \
             tc.tile_pool(name="persist", bufs=1) as persist, \
             tc.tile_pool(name="drampool", bufs=1, space="DRAM") as drampool:
            # u_hat, fp32r, [c_part, chunk, b, n, s]
            A = apool.tile([128, NCHUNK, BL, NUM_UNIT, UNIT_SIZE], f32r)
            Aap = A[:]
            pstA = Aap.ap[0][0]
            b_ij = persist.tile([128, NCHUNK, NUM_UNIT], f32)
            cij_u = persist.tile([128, NUM_UNIT], f32r)   # uniform 1/16
            nc.gpsimd.memset(b_ij[:], 0.0)
            nc.sync.dma_start(cij_u[:], cij1_t[:].bitcast(f32r))

            # ---------------- production ----------------
            with tc.tile_pool(name="bdp", bufs=2) as bdp, \
                 tc.tile_pool(name="wp", bufs=3) as wp, \
                 tc.tile_pool(name="stgp", bufs=3) as stgp, \
                 tc.tile_pool(name="psp", bufs=4, space="PSUM") as psp:
                for sg in range(NCHUNK):        # one c-chunk = 16 groups
                    quads = []
                    for q in range(4):
                        bd = bdp.tile([128, 4, 128], f32r, tag=f"bd{q}",
                                      name=f"bd{q}_{sg}")
                        nc.gpsimd.memset(bd[:].bitcast(f32), 0.0)
                        pstB = bd[:].ap[0][0]
                        for cc in range(8):
                            # block-diag xT: bd[(cc,i), gq, cc*16+b]
                            src = sub(xc_t[:], (8 * (16 * sg + 4 * q) + cc)
                                      * IN_UNIT * BL,
                                      [[BL, IN_UNIT],
                                       [8 * IN_UNIT * BL, 4],
                                       [1, BL]], cast=f32r)
                            dst = sub(bd[:], cc * 16 * pstB + cc * 16,
                                      [[pstB, IN_UNIT], [128, 4], [1, BL]])
                            nc.sync.dma_start(dst, src)
                        quads.append(bd)
                    for gg in range(16):
                        g = sg * 16 + gg
                        wt = wp.tile([128, 256], f32r, tag="w", name=f"w_{g}")
                        nc.sync.dma_start(wt[:], wr_t[g * 128:(g + 1) * 128, :]
                                          .bitcast(f32r))
                        ps = psp.tile([128, 256], f32, tag="ps", name=f"ps_{g}")
                        bdq = quads[gg // 4]
                        nc.tensor.matmul(ps[:], bdq[:, gg % 4, :], wt[:],
                                         start=True, stop=True)
                        stg = stgp.tile([128, 256], f32, tag="stg",
                                        name=f"stg_{g}")
                        if gg % 2 == 0:
                            nc.vector.tensor_copy(stg[:], ps[:])
                        else:
                            nc.scalar.copy(stg[:], ps[:])
                        # shuffle (cc*16+b),(n,s) -> part 8gg+cc, (b,n,s)
                        dstA = sub(Aap, 8 * gg * pstA + sg * FREE,
                                   [[pstA, 8], [NS, BL], [1, NS]], cast=f32)
                        nc.sync.dma_start(dstA, stg[:])

            # ---------------- routing ----------------
            with tc.tile_pool(name="rt", bufs=1) as rt, \
                 tc.tile_pool(name="tb", bufs=2) as tb, \
                 tc.tile_pool(name="pss", bufs=1, space="PSUM") as pss:
                vb = rt.tile([128, FREE], f32)
                cij = rt.tile([128, NCHUNK, NUM_UNIT], f32)
                cij_r = rt.tile([128, NCHUNK, NUM_UNIT], f32r)
                smax = rt.tile([128, NCHUNK], f32)
                ssum = rt.tile([128, NCHUNK], f32)
                uv = rt.tile([128, NCHUNK, NUM_UNIT], f32)
                ar_sb = rt.tile([128, NCHUNK, NUM_UNIT], f32)

                for it in range(3):
                    if it > 0:
                        # softmax over n of b_ij -> cij (f32r via DMA recast)
                        nc.vector.tensor_reduce(smax[:], b_ij[:], axis=AX.X,
                                                op=ALU.max)
                        mb = sub(smax[:], 0,
                                 [[NCHUNK, 128], [1, NCHUNK], [0, NUM_UNIT]])
                        nc.vector.tensor_tensor(cij[:], b_ij[:], mb,
                                                op=ALU.subtract)
                        nc.scalar.activation(cij[:], cij[:],
                                             mybir.ActivationFunctionType.Exp)
                        nc.vector.tensor_reduce(ssum[:], cij[:], axis=AX.X,
                                                op=ALU.add)
                        nc.vector.reciprocal(ssum[:], ssum[:])
                        sb = sub(ssum[:], 0,
                                 [[NCHUNK, 128], [1, NCHUNK], [0, NUM_UNIT]])
                        nc.vector.tensor_tensor(cij[:], cij[:], sb, op=ALU.mult)
                        nc.sync.dma_start(cij_r[:], cij[:].bitcast(f32r))

                    # s_j: PE pass, c_ij stationary, u_hat moving
                    psj = pss.tile([NUM_UNIT, FREE], f32, tag="psj",
                                   name=f"psj_{it}")
                    pstP = psj[:].ap[0][0]
                    for k in range(NCHUNK):
                        lhs = cij_u[:] if it == 0 else cij_r[:, k, :]
                        for j in range(FREE // 512):
                            nc.tensor.matmul(
                                psj[:, j * 512:(j + 1) * 512], lhs,
                                sub(Aap, k * FREE + j * 512,
                                    [[pstA, 128], [1, 512]]),
                                start=(k == 0), stop=(k == NCHUNK - 1))

                    # diagonal extract: s[n,(b,s)] = psj[n, (b,n,s)]
                    sjf = tb.tile([NUM_UNIT, FREE], f32, tag="big1", bufs=1,
                                  name=f"sjf_{it}")
                    nc.vector.tensor_copy(sjf[:], psj[:])
                    pstS = sjf[:].ap[0][0]
                    s_t = tb.tile([NUM_UNIT, BL, UNIT_SIZE], f32, tag="s_t",
                                  name=f"s_t{it}")
                    pstST = s_t[:].ap[0][0]
                    for n in range(NUM_UNIT):
                        src = sub(sjf[:], n * pstS + n * UNIT_SIZE,
                                  [[pstS, 1], [NS, BL], [1, UNIT_SIZE]])
                        dst = sub(s_t[:], n * pstST,
                                  [[pstST, 1], [UNIT_SIZE, BL],
                                   [1, UNIT_SIZE]])
                        nc.sync.dma_start(dst, src)

                    # squash over s
                    s2 = tb.tile([NUM_UNIT, BL, UNIT_SIZE], f32, tag="big1",
                                 bufs=1, name=f"s2_{it}")
                    nc.vector.tensor_tensor(s2[:], s_t[:], s_t[:], op=ALU.mult)
                    sq = tb.tile([NUM_UNIT, BL], f32, tag="sq", name=f"sq_{it}")
                    nc.vector.tensor_reduce(sq[:], s2[:], axis=AX.X, op=ALU.add)
                    rsq = tb.tile([NUM_UNIT, BL], f32, tag="rsq",
                                  name=f"rsq_{it}")
                    nc.scalar.sqrt(rsq[:], sq[:])
                    den = tb.tile([NUM_UNIT, BL], f32, tag="den",
                                  name=f"den_{it}")
                    nc.vector.scalar_tensor_tensor(den[:], sq[:], 1.0, rsq[:],
                                                   op0=ALU.add, op1=ALU.mult)
                    nc.vector.reciprocal(den[:], den[:])
                    fac = tb.tile([NUM_UNIT, BL], f32, tag="fac",
                                  name=f"fac_{it}")
                    nc.vector.tensor_tensor(fac[:], sq[:], den[:], op=ALU.mult)
                    v_t = tb.tile([NUM_UNIT, BL, UNIT_SIZE], f32, tag="v_t",
                                  name=f"v_t{it}")
                    pstF = fac[:].ap[0][0]
                    fb = sub(fac[:], 0, [[pstF, NUM_UNIT], [1, BL],
                                         [0, UNIT_SIZE]])
                    nc.vector.tensor_tensor(v_t[:], s_t[:], fb, op=ALU.mult)

                    if it == 2:
                        nc.sync.dma_start(vout_t[:],
                                          sub(v_t[:], 0,
                                              [[v_t[:].ap[0][0], NUM_UNIT],
                                               [1, BL * UNIT_SIZE]]))
                        break

                    # flatten v[n,(b,s)] -> vb[0, (b,n,s)], one DMA per n
                    pstV = v_t[:].ap[0][0]
                    pstVB = vb[:].ap[0][0]
                    for n in range(NUM_UNIT):
                        dstv = sub(vb[:], n * UNIT_SIZE,
                                   [[pstVB, 1], [NS, BL], [1, UNIT_SIZE]])
                        srcv = sub(v_t[:], n * pstV,
                                   [[pstV, 1], [UNIT_SIZE, BL],
                                    [1, UNIT_SIZE]])
                        nc.sync.dma_start(dstv, srcv)
                    nc.gpsimd.partition_broadcast(vb[:, :], vb[0:1, :])

                    # agreement: uv[c,n] = sum_{b,s} u_hat * v
                    QF = FREE // 4          # 1024 = 4 batches
                    for k in range(NCHUNK):
                        rsb = tb.tile([128, 4, 4, NUM_UNIT], f32, tag="rsb",
                                      name=f"rsb_{it}_{k}")
                        for h in range(4):
                            tmp = tb.tile([128, QF], f32, tag="uvt",
                                          name=f"uvt_{it}_{k}_{h}")
                            nc.gpsimd.tensor_tensor(
                                tmp[:],
                                sub(Aap, k * FREE + h * QF,
                                    [[pstA, 128], [1, QF]], cast=f32),
                                vb[:, h * QF:(h + 1) * QF],
                                op=ALU.mult)
                            pstT = tmp[:].ap[0][0]
                            nc.vector.tensor_reduce(
                                rsb[:, h],
                                sub(tmp[:], 0,
                                    [[pstT, 128], [NS, 4],
                                     [UNIT_SIZE, NUM_UNIT], [1, UNIT_SIZE]]),
                                axis=AX.X, op=ALU.add)
                        pstR = rsb[:].ap[0][0]
                        nc.vector.tensor_reduce(
                            uv[:, k], sub(rsb[:], 0,
                                          [[pstR, 128], [1, NUM_UNIT],
                                           [NUM_UNIT, 16]]),
                            axis=AX.X, op=ALU.add)

                    arbounce_i = drampool.tile([128, NCHUNK * NUM_UNIT], f32,
                                               name=f"arbi_{it}", tag=f"arbi{it}")
                    arbounce_o = drampool.tile([128, NCHUNK * NUM_UNIT], f32,
                                               addr_space="Shared",
                                               name=f"arbo_{it}", tag=f"arbo{it}")
                    nc.gpsimd.dma_start(arbounce_i[:], uv[:])
                    nc.gpsimd.collective_compute(
                        "AllReduce", ALU.add,
                        replica_groups=[list(range(NCORES))],
                        ins=[arbounce_i.opt()], outs=[arbounce_o.opt()])
                    nc.sync.dma_start(ar_sb[:], arbounce_o[:])
                    # b_ij += AR/B
                    nc.vector.scalar_tensor_tensor(b_ij[:], ar_sb[:], 1.0 / B,
                                                   b_ij[:], op0=ALU.mult,
                                                   op1=ALU.add)

    nc.compile()
    return nc


def _prep(x, weight):
    wr = np.ascontiguousarray(
        weight.reshape(NGROUP, 8, NUM_UNIT, UNIT_SIZE, IN_UNIT)
        .transpose(0, 1, 4, 2, 3).reshape(NGROUP * 128, 256)).astype(np.float32)
    cij1 = np.full((128, NUM_UNIT), 1.0 / NUM_UNIT, np.float32)
    in_maps = []
    for c in range(NCORES):
        xs = x[c * BL:(c + 1) * BL]          # [BL, i, C]
        xc = np.ascontiguousarray(xs.transpose(2, 1, 0)).astype(np.float32)
        in_maps.append({"wr": wr, "xc": xc, "cij1": cij1})
    return in_maps


def kernel(x, x_original, weight, mode, epoch, _trace=False):
    from concourse.bass_utils import run_bass_kernel_spmd

    x = np.asarray(x, dtype=np.float32)
    weight = np.asarray(weight, dtype=np.float32)
    if "nc" not in _cache:
        _cache["nc"] = _build()
    nc = _cache["nc"]
    in_maps = _prep(x, weight)
    res = run_bass_kernel_spmd(nc, in_maps, core_ids=list(range(NCORES)),
                               trace=_trace)
    _cache["last_result"] = res
    out = np.empty((B, NUM_UNIT, UNIT_SIZE), np.float32)
    for c in range(NCORES):
        vo = res.results[c]["vout"].reshape(NUM_UNIT, BL, UNIT_SIZE)
        out[c * BL:(c + 1) * BL] = vo.transpose(1, 0, 2)
    return out[..., None]


# revision 22
# speedup vs baseline: 142.3223x; 142.3223x over previous
"""CapsuleLayer dynamic-routing kernel for 8 Trainium2 NeuronCores.

Sharding: data-parallel over batch (16 batches/core), weight replicated.
  u_hat[b,c,n,s] = sum_i W[c,n,s,i] * x[b,i,c]   (PE, fp32r, block-diag x)
  3 routing iterations; the b_ij update takes a mean over the full batch
  via one AllReduce per iteration (skipped on the last).

On-chip: u_hat kept in SBUF as A[c%128, chunk, b, n, s] (9 chunks of 128
channels).  s_j is a PE pass (c_ij stationary, u_hat moving); the
agreement <u_hat, v> is a GPSIMD multiply + DVE segmented reduce.
"""

import sys

sys.path.insert(0, "/opt/trn_rl_repo")

import numpy as np

B, IN_UNIT, IN_CHANNEL = 128, 16, 1152
NUM_UNIT, UNIT_SIZE = 16, 16
NCORES = 8
BL = B // NCORES               # 16 batches per core
NGROUP = IN_CHANNEL // 8       # 144 groups of 8 channels
NCHUNK = IN_CHANNEL // 128     # 9 c-chunks
NS = NUM_UNIT * UNIT_SIZE      # 256
FREE = BL * NS                 # 4096 = (b, n, s) free size per chunk

_cache = {}


def _build(single_core=False, niters=3, skip_prod=False):
    import concourse.bass as bass
    import concourse.bacc as bacc
    import concourse.mybir as mybir
    import concourse.tile as tile

    f32 = mybir.dt.float32
    f32r = mybir.dt.float32r
    ALU = mybir.AluOpType
    AX = mybir.AxisListType

    def sub(ap, off, dims, cast=None):
        a = bass.AP(ap.tensor, ap.offset + off, [list(d) for d in dims])
        return a.bitcast(cast) if cast is not None else a

    nc = bacc.Bacc("TRN2", target_bir_lowering=False, debug=False,
                   num_devices=1 if single_core else NCORES)

    wr_t = nc.dram_tensor("wr", [NGROUP * 128, 256], f32, kind="ExternalInput")
    xc_t = nc.dram_tensor("xc", [IN_CHANNEL, IN_UNIT, BL], f32, kind="ExternalInput")
    cij1_t = nc.dram_tensor("cij1", [128, NUM_UNIT], f32, kind="ExternalInput")
    vout_t = nc.dram_tensor("vout", [NUM_UNIT, BL * UNIT_SIZE], f32,
                            kind="ExternalOutput")

    with tile.TileContext(nc) as tc:
        with tc.tile_pool(name="apool", bufs=1) as apool, \
             tc.tile_pool(name="persist", bufs=1) as persist, \
             tc.tile_pool(name="drampool", bufs=1, space="DRAM") as drampool:
            # u_hat, fp32r, [c_part, chunk, b, n, s]
            A = apool.tile([128, NCHUNK, BL, NUM_UNIT, UNIT_SIZE], f32r)
            Aap = A[:]
            pstA = Aap.ap[0][0]
            b_ij = persist.tile([128, NCHUNK, NUM_UNIT], f32)
            cij_u = persist.tile([128, NUM_UNIT], f32r)   # uniform 1/16
            nc.gpsimd.memset(b_ij[:], 0.0)
            nc.sync.dma_start(cij_u[:], cij1_t[:].bitcast(f32r))

            # ---------------- production ----------------
            uhd = drampool.tile([NGROUP * 128, 256], f32)    # u_hat bounce
            with tc.tile_pool(name="bdp", bufs=1) as bdp, \
                 tc.tile_pool(name="wp", bufs=1) as wp, \
                 tc.tile_pool(name="stgp", bufs=1) as stgp, \
                 tc.tile_pool(name="psp", bufs=8, space="PSUM") as psp:
                for sg in (range(NCHUNK) if not skip_prod else []):
                    bd16 = bdp.tile([128, 16, 128], f32r, tag="bd16",
                                    name=f"bd16_{sg}")
                    pstB = bd16[:].ap[0][0]
                    if sg < 1:      # single slot; zero padding persists
                        nc.gpsimd.memset(bd16[:].bitcast(f32), 0.0)
                    for cc in range(8):
                        # block-diag xT: bd16[(cc,i), g, cc*16+b]
                        src = sub(xc_t[:], (8 * 16 * sg + cc) * IN_UNIT * BL,
                                  [[BL, IN_UNIT],
                                   [8 * IN_UNIT * BL, 16],
                                   [1, BL]], cast=f32r)
                        dst = sub(bd16[:], cc * 16 * pstB + cc * 16,
                                  [[pstB, IN_UNIT], [128, 16], [1, BL]])
                        nc.sync.dma_start(dst, src)
                    wts = []
                    for gq in range(4):
                        wt4 = wp.tile([128, 4, 256], f32r, tag=f"w{gq}",
                                      name=f"w{gq}_{sg}")
                        nc.sync.dma_start(
                            wt4[:], sub(wr_t[:], (sg * 16 + gq * 4) * 128 * 256,
                                        [[256, 128], [128 * 256, 4], [1, 256]],
                                        cast=f32r))
                        wts.append(wt4)
                    stgb = stgp.tile([128, 16, 256], f32, tag="stgb",
                                     name=f"stgb_{sg}")
                    for gg in range(16):
                        ps = psp.tile([128, 256], f32, tag="ps",
                                      name=f"ps_{sg}_{gg}")
                        nc.tensor.matmul(ps[:], bd16[:, gg, :],
                                         wts[gg // 4][:, gg % 4, :],
                                         start=True, stop=True)
                        if gg % 2 == 0:
                            nc.vector.tensor_copy(stgb[:, gg, :], ps[:])
                        else:
                            nc.scalar.copy(stgb[:, gg, :], ps[:])
                    # (cc,b),(g,n,s) -> DRAM uhd[(g,cc,b), (n,s)]
                    dstu = sub(uhd[:], sg * 16 * 128 * 256,
                               [[16 * 256, 8], [256, 16],
                                [8 * 16 * 256, 16], [1, 256]])
                    nc.sync.dma_start(dstu, stgb[:])
                    # readback c-partitioned: A[p, sg, (b,n,s)]
                    dstA = sub(Aap, sg * FREE, [[pstA, 128], [1, FREE]],
                               cast=f32)
                    srcu = sub(uhd[:], sg * 16 * 128 * 256,
                               [[FREE, 128], [1, FREE]])
                    nc.sync.dma_start(dstA, srcu)

            # ---------------- routing ----------------
            with tc.tile_pool(name="rt", bufs=1) as rt, \
                 tc.tile_pool(name="tb", bufs=2) as tb, \
                 tc.tile_pool(name="pss", bufs=1, space="PSUM") as pss:
                vb = rt.tile([128, FREE], f32)
                cij = rt.tile([128, NCHUNK, NUM_UNIT], f32)
                cij_r = rt.tile([128, NCHUNK, NUM_UNIT], f32r)
                smax = rt.tile([128, NCHUNK], f32)
                ssum = rt.tile([128, NCHUNK], f32)
                uv = rt.tile([128, NCHUNK, NUM_UNIT], f32)
                ar_sb = rt.tile([128, NCHUNK, NUM_UNIT], f32)

                for it in range(niters):
                    if it > 0:
                        # softmax over n of b_ij -> cij (f32r via DMA recast)
                        nc.vector.tensor_reduce(smax[:], b_ij[:], axis=AX.X,
                                                op=ALU.max)
                        mb = sub(smax[:], 0,
                                 [[NCHUNK, 128], [1, NCHUNK], [0, NUM_UNIT]])
                        nc.vector.tensor_tensor(cij[:], b_ij[:], mb,
                                                op=ALU.subtract)
                        nc.scalar.activation(cij[:], cij[:],
                                             mybir.ActivationFunctionType.Exp)
                        nc.vector.tensor_reduce(ssum[:], cij[:], axis=AX.X,
                                                op=ALU.add)
                        nc.vector.reciprocal(ssum[:], ssum[:])
                        sb = sub(ssum[:], 0,
                                 [[NCHUNK, 128], [1, NCHUNK], [0, NUM_UNIT]])
                        nc.vector.tensor_tensor(cij[:], cij[:], sb, op=ALU.mult)
                        nc.sync.dma_start(cij_r[:], cij[:].bitcast(f32r))

                    # s_j: PE pass, c_ij stationary, u_hat moving
                    psj = pss.tile([NUM_UNIT, FREE], f32, tag="psj",
                                   name=f"psj_{it}")
                    pstP = psj[:].ap[0][0]
                    for k in range(NCHUNK):
                        lhs = cij_u[:] if it == 0 else cij_r[:, k, :]
                        for j in range(FREE // 512):
                            nc.tensor.matmul(
                                psj[:, j * 512:(j + 1) * 512], lhs,
                                sub(Aap, k * FREE + j * 512,
                                    [[pstA, 128], [1, 512]]),
                                start=(k == 0), stop=(k == NCHUNK - 1))

                    # diagonal extract: s[n,(b,s)] = psj[n, (b,n,s)]
                    sjf = tb.tile([NUM_UNIT, FREE], f32, tag="big1", bufs=1,
                                  name=f"sjf_{it}")
                    nc.vector.tensor_copy(sjf[:, :FREE // 2],
                                          psj[:, :FREE // 2])
                    nc.scalar.copy(sjf[:, FREE // 2:], psj[:, FREE // 2:])
                    pstS = sjf[:].ap[0][0]
                    s_t = tb.tile([NUM_UNIT, BL, UNIT_SIZE], f32, tag="s_t",
                                  name=f"s_t{it}")
                    pstST = s_t[:].ap[0][0]
                    # one DMA: partition-dim step carries the diagonal offset
                    src = sub(sjf[:], 0,
                              [[pstS + UNIT_SIZE, NUM_UNIT],
                               [NS, BL], [1, UNIT_SIZE]])
                    nc.sync.dma_start(s_t[:], src)

                    # squash over s
                    s2 = tb.tile([NUM_UNIT, BL, UNIT_SIZE], f32, tag="big1",
                                 bufs=1, name=f"s2_{it}")
                    nc.vector.tensor_tensor(s2[:], s_t[:], s_t[:], op=ALU.mult)
                    sq = tb.tile([NUM_UNIT, BL], f32, tag="sq", name=f"sq_{it}")
                    nc.vector.tensor_reduce(sq[:], s2[:], axis=AX.X, op=ALU.add)
                    rsq = tb.tile([NUM_UNIT, BL], f32, tag="rsq",
                                  name=f"rsq_{it}")
                    nc.scalar.sqrt(rsq[:], sq[:])
                    den = tb.tile([NUM_UNIT, BL], f32, tag="den",
                                  name=f"den_{it}")
                    nc.vector.scalar_tensor_tensor(den[:], sq[:], 1.0, rsq[:],
                                                   op0=ALU.add, op1=ALU.mult)
                    nc.vector.reciprocal(den[:], den[:])
                    fac = tb.tile([NUM_UNIT, BL], f32, tag="fac",
                                  name=f"fac_{it}")
                    nc.vector.tensor_tensor(fac[:], sq[:], den[:], op=ALU.mult)
                    v_t = tb.tile([NUM_UNIT, BL, UNIT_SIZE], f32, tag="v_t",
                                  name=f"v_t{it}")
                    pstF = fac[:].ap[0][0]
                    fb = sub(fac[:], 0, [[pstF, NUM_UNIT], [1, BL],
                                         [0, UNIT_SIZE]])
                    nc.vector.tensor_tensor(v_t[:], s_t[:], fb, op=ALU.mult)

                    if it == niters - 1:
                        nc.sync.dma_start(vout_t[:],
                                          sub(v_t[:], 0,
                                              [[v_t[:].ap[0][0], NUM_UNIT],
                                               [1, BL * UNIT_SIZE]]))
                        break

                    # flatten v[n,(b,s)] -> vb[0, (b,n,s)], one DMA per n
                    pstV = v_t[:].ap[0][0]
                    pstVB = vb[:].ap[0][0]
                    for n in range(NUM_UNIT):
                        dstv = sub(vb[:], n * UNIT_SIZE,
                                   [[pstVB, 1], [NS, BL], [1, UNIT_SIZE]])
                        srcv = sub(v_t[:], n * pstV,
                                   [[pstV, 1], [UNIT_SIZE, BL],
                                    [1, UNIT_SIZE]])
                        nc.sync.dma_start(dstv, srcv)
                    nc.gpsimd.partition_broadcast(vb[:, :], vb[0:1, :])

                    # agreement: uv[c,n] = sum_{b,s} u_hat * v
                    QF = FREE // 4          # 1024 = 4 batches
                    for k in range(NCHUNK):
                        rsb = tb.tile([128, 4, 4, NUM_UNIT], f32, tag="rsb",
                                      name=f"rsb_{it}_{k}")
                        for h in range(4):
                            tmp = tb.tile([128, QF], f32, tag="uvt",
                                          name=f"uvt_{it}_{k}_{h}")
                            eng = (nc.gpsimd if (k * 4 + h) % 5 < 3
                                   else nc.vector)
                            eng.tensor_tensor(
                                tmp[:],
                                sub(Aap, k * FREE + h * QF,
                                    [[pstA, 128], [1, QF]], cast=f32),
                                vb[:, h * QF:(h + 1) * QF],
                                op=ALU.mult)
                            pstT = tmp[:].ap[0][0]
                            nc.vector.tensor_reduce(
                                rsb[:, h],
                                sub(tmp[:], 0,
                                    [[pstT, 128], [NS, 4],
                                     [UNIT_SIZE, NUM_UNIT], [1, UNIT_SIZE]]),
                                axis=AX.X, op=ALU.add)
                        pstR = rsb[:].ap[0][0]
                        nc.vector.tensor_reduce(
                            uv[:, k], sub(rsb[:], 0,
                                          [[pstR, 128], [1, NUM_UNIT],
                                           [NUM_UNIT, 16]]),
                            axis=AX.X, op=ALU.add)

                    arbounce_i = drampool.tile([128, NCHUNK * NUM_UNIT], f32,
                                               name=f"arbi_{it}", tag=f"arbi{it}")
                    arbounce_o = drampool.tile([128, NCHUNK * NUM_UNIT], f32,
                                               addr_space="Shared",
                                               name=f"arbo_{it}", tag=f"arbo{it}")
                    nc.gpsimd.dma_start(arbounce_i[:], uv[:])
                    if single_core:
                        nc.gpsimd.dma_start(arbounce_o[:], arbounce_i[:])
                    else:
                        nc.gpsimd.collective_compute(
                            "AllReduce", ALU.add,
                            replica_groups=[list(range(NCORES))],
                            ins=[arbounce_i.opt()], outs=[arbounce_o.opt()])
                    nc.sync.dma_start(ar_sb[:], arbounce_o[:])
                    # b_ij += AR/B
                    nc.vector.scalar_tensor_tensor(b_ij[:], ar_sb[:], 1.0 / B,
                                                   b_ij[:], op0=ALU.mult,
                                                   op1=ALU.add)

    nc.compile()
    return nc


def _prep(x, weight):
    wr = np.ascontiguousarray(
        weight.reshape(NGROUP, 8, NUM_UNIT, UNIT_SIZE, IN_UNIT)
        .transpose(0, 1, 4, 2, 3).reshape(NGROUP * 128, 256)).astype(np.float32)
    cij1 = np.full((128, NUM_UNIT), 1.0 / NUM_UNIT, np.float32)
    in_maps = []
    for c in range(NCORES):
        xs = x[c * BL:(c + 1) * BL]          # [BL, i, C]
        xc = np.ascontiguousarray(xs.transpose(2, 1, 0)).astype(np.float32)
        in_maps.append({"wr": wr, "xc": xc, "cij1": cij1})
    return in_maps


def kernel(x, x_original, weight, mode, epoch, _trace=False):
    from concourse.bass_utils import run_bass_kernel_spmd

    x = np.asarray(x, dtype=np.float32)
    weight = np.asarray(weight, dtype=np.float32)
    if "nc" not in _cache:
        _cache["nc"] = _build()
    nc = _cache["nc"]
    in_maps = _prep(x, weight)
    res = run_bass_kernel_spmd(nc, in_maps, core_ids=list(range(NCORES)),
                               trace=_trace)
    _cache["last_result"] = res
    out = np.empty((B, NUM_UNIT, UNIT_SIZE), np.float32)
    for c in range(NCORES):
        vo = res.results[c]["vout"].reshape(NUM_UNIT, BL, UNIT_SIZE)
        out[c * BL:(c + 1) * BL] = vo.transpose(1, 0, 2)
    return out[..., None]
